# revision 36
# baseline (speedup 1.0000x reference)
"""Multi-head attention (BaselineAttention) Bass kernel for 8 trn2 NeuronCores.

Problem: x[4,2048,1024], per-head Wq/Wk/Wv [16,1024,64] (+biases), Wo[1024,1024]+bo.
Sharding: core c -> batch b=c//2, head-group g=c%2 (8 heads each).
Each core computes y_partial[b] = sum_{h in group} softmax(qk^T/8) v @ Wo_rows(h).
Host combines: y[b] = part[2b] + part[2b+1] + bo + bv@Wo  (bv folded out of device).

Device algorithm per core (all matmul operands bf16; psum f32):
  x resident in SBUF [128, 8kt, 2048]; per pair (2 heads): qT/kT [128=(j,e), s]
  via W^T@x; v[t,(j,e)] via x^T@Wv with an appended ones column (vA [.,tt,j,65]).
  Attention per (head, s-chunk 512): scores^T[t,s] = kT^T qT (K=64); exp on ACT
  -> att bf16 [t, s]; o_aug[s, e|r] = att^T-stationary @ [v|1]-moving (N=65,
  psum-accumulated over 16 t-tiles); normalize with DVE per-partition scalar
  1/r; PE-transpose o_n -> onorm[(j,e), pair, s]; out-proj y = onorm^T @ Wo.
  Pair-pipelined schedule: QKV of pair p+1 and out-proj fill PE while ACT
  computes exp of pair p (exp is the co-bottleneck at ~267us vs PE ~285us).
"""
import numpy as np

B, S, DIM, H, DH = 4, 2048, 1024, 16, 64
NCORES = 8
HPC = H // 2          # heads per core = 8
NPAIR = HPC // 2      # head pairs per core = 4
NT = S // 128         # t-tiles = 16
NSQ = S // 512        # s-chunks of 512 = 4
NKT = DIM // 128      # d-tiles = 8
SCALE = 1.0 / float(np.sqrt(DH))

_CACHE = {}


def _build(repeat=1, debug_taps=False, exp_ns=1040, attnv_ns=1830, defer_q=True, greedy_m=-600):
    from collections import deque
    import concourse.bass as bass  # noqa: F401
    import concourse.mybir as mybir
    import concourse.tile as tile
    from concourse import bacc

    f32 = mybir.dt.float32
    bf16 = mybir.dt.bfloat16
    AF = mybir.ActivationFunctionType

    nc = bacc.Bacc("TRN2", target_bir_lowering=False, debug=False,
                   num_devices=NCORES)

    xT_d = nc.dram_tensor("xT", [DIM, S], bf16, kind="ExternalInput")
    wq_d = nc.dram_tensor("wq", [NPAIR, 128, NKT, 128], bf16, kind="ExternalInput")
    wk_d = nc.dram_tensor("wk", [NPAIR, 128, NKT, 128], bf16, kind="ExternalInput")
    wv_d = nc.dram_tensor("wv", [NPAIR, 128, NKT, 128], bf16, kind="ExternalInput")
    bq_d = nc.dram_tensor("bq", [128, NPAIR], f32, kind="ExternalInput")
    bk_d = nc.dram_tensor("bk", [128, NPAIR], f32, kind="ExternalInput")
    wo_d = nc.dram_tensor("wo", [128, NPAIR, DIM], bf16, kind="ExternalInput")
    eye_d = nc.dram_tensor("eye", [128, 128], bf16, kind="ExternalInput")
    y_d = nc.dram_tensor("y", [S, DIM], f32, kind="ExternalOutput")
    if debug_taps:
        dbg_qT = nc.dram_tensor("dbg_qT", [128, S], bf16, kind="ExternalOutput")
        dbg_kT = nc.dram_tensor("dbg_kT", [128, S], bf16, kind="ExternalOutput")
        dbg_vA = nc.dram_tensor("dbg_vA", [128, NT, 2, 65], bf16,
                                kind="ExternalOutput")
        dbg_att = nc.dram_tensor("dbg_att", [128, NT, 512], bf16,
                                 kind="ExternalOutput")
        dbg_on = nc.dram_tensor("dbg_on", [128, NPAIR, S], bf16,
                                kind="ExternalOutput")

    ctr = [0]

    def nm(pfx):
        ctr[0] += 1
        return f"{pfx}_{ctr[0]}"

    with tile.TileContext(nc) as tc:
        with tc.tile_pool(name="persist", bufs=1) as pp, \
             tc.tile_pool(name="work", bufs=1) as wp, \
             tc.tile_pool(name="ps", bufs=1, space="PSUM") as ps:
            # ---- persistent SBUF ----
            x_sb = pp.tile([128, NKT, S], bf16)
            eye = pp.tile([128, 128], bf16)
            bqs = pp.tile([128, NPAIR], f32)
            bks = pp.tile([128, NPAIR], f32)
            wo_sb = pp.tile([128, NPAIR, DIM], bf16)
            onorm = pp.tile([128, NPAIR, S], bf16)   # [(j,e), pair, s]

            for rep in range(repeat):
                xT_src = xT_d.ap().rearrange("(kt p) s -> p kt s", p=128)
                nc.sync.dma_start(out=x_sb[:, 0, :], in_=xT_src[:, 0, :])

                wtiles = {}

                def w_dmas(p, only=None):
                    for pfx, wd in (("k", wk_d), ("q", wq_d), ("v", wv_d)):
                        if only and pfx not in only:
                            continue
                        w = wp.tile([128, NKT, 128], bf16, tag=f"w{pfx}",
                                    bufs=2, name=nm(f"w{pfx}{p}"))
                        nc.sync.dma_start(out=w, in_=wd.ap()[p])
                        wtiles[(pfx, p)] = w

                qkts = {}

                def pair_tiles(p):
                    qT = wp.tile([128, S], bf16, tag="qT", bufs=2,
                                 name=nm(f"qT{p}"))
                    kT = wp.tile([128, S], bf16, tag="kT", bufs=2,
                                 name=nm(f"kT{p}"))
                    vA = wp.tile([128, NT, 2, 65], bf16, tag="vA", bufs=2,
                                 name=nm(f"vA{p}"))
                    qkts[p] = (qT, kT, vA)

                def ones_atom(p):
                    def go():
                        nc.gpsimd.memset(qkts[p][2][:, :, :, 64:65], 1.0)
                    return go

                def qk_atoms(pfx, p, sq):
                    """Two ~850ns PE atoms (4 kt-matmuls each) + bias copy."""
                    cell = {}

                    def a1():
                        dst, bias = ((qkts[p][0], bqs) if pfx == "q"
                                     else (qkts[p][1], bks))
                        cell["ps"] = ps.tile([128, 512], f32, tag="qk",
                                             bufs=1, name=nm(f"ps{pfx}{p}{sq}"))
                        cell["dst"], cell["bias"] = dst, bias
                        w = wtiles[(pfx, p)]
                        for kt in range(4):
                            nc.tensor.matmul(
                                cell["ps"], w[:, kt, :],
                                x_sb[:, kt, sq * 512:(sq + 1) * 512],
                                start=(kt == 0), stop=False)

                    def a2():
                        w = wtiles[(pfx, p)]
                        for kt in range(4, NKT):
                            nc.tensor.matmul(
                                cell["ps"], w[:, kt, :],
                                x_sb[:, kt, sq * 512:(sq + 1) * 512],
                                start=False, stop=(kt == NKT - 1))
                        nc.vector.tensor_scalar_add(
                            out=cell["dst"][:, sq * 512:(sq + 1) * 512],
                            in0=cell["ps"], scalar1=cell["bias"][:, p:p + 1])
                    return [a1, a2]

                def v_atoms(p, ttg):
                    """Two ~850ns PE atoms (2 t-tiles each) + copy to vA."""
                    cell = {}

                    def half(h):
                        def go():
                            if h == 0:
                                cell["ps"] = ps.tile(
                                    [128, 512], f32, tag="vps", bufs=1,
                                    name=nm(f"psv{p}{ttg}"))
                            v_ps4 = cell["ps"].rearrange("p (t m) -> p t m",
                                                         t=4)
                            w = wtiles[("v", p)]
                            for t4 in (0, 1) if h == 0 else (2, 3):
                                tt = ttg * 4 + t4
                                for kt in range(NKT):
                                    nc.tensor.matmul(
                                        v_ps4[:, t4, :],
                                        x_sb[:, kt, tt * 128:(tt + 1) * 128],
                                        w[:, kt, :],
                                        start=(kt == 0), stop=(kt == NKT - 1))
                            if h == 1:
                                vA = qkts[p][2]
                                nc.vector.tensor_copy(
                                    vA[:, ttg * 4:(ttg + 1) * 4, :, 0:64],
                                    cell["ps"].rearrange(
                                        "p (t j e) -> p t j e", t=4, j=2))
                        return go
                    return [half(0), half(1)]

                def qkv_atoms(p):
                    """(resource_key, cost_ns, closure) atoms for pair p."""
                    pair_tiles(p)
                    out = [((p, "v"), 100, ones_atom(p))]
                    for ttg in range(4):
                        out += [((p, "v"), 880, a) for a in v_atoms(p, ttg)]
                    for sq in range(NSQ):
                        out += [((p, "k"), 880, a)
                                for a in qk_atoms("k", p, sq)]
                    for sq in range(NSQ):
                        out += [((p, f"q{sq}"), 880, a)
                                for a in qk_atoms("q", p, sq)]
                    return out

                # ---- greedy emission scheduler ----
                # Estimated engine clocks (ns) maintained during emission;
                # cold filler (QKV / out-proj atoms) is emitted only while
                # it cannot starve ACT of score tiles.
                GREEDY_M = greedy_m
                SC_MM = 217       # one [128,512] scores matmul
                EXP = exp_ns      # one [128,1024] exp + init
                ATTNV = attnv_ns  # 64 x 65-col matmuls
                TRANSP = 320
                est = {"pe": 0.0, "act": 0.0}
                exp_hist = [0.0, 0.0]   # completion time per sc-tile rotation

                hot = deque()
                cold = deque()

                def pop_cold_one():
                    key, cost, fn = cold.popleft()
                    fn()
                    est["pe"] += cost

                def force_res(*keys):
                    """Emit every cold atom up to and incl. the given
                    resource keys (FIFO order preserved)."""
                    while any(k for (k, c, f) in cold if k in keys):
                        pop_cold_one()

                def greedy_cold():
                    while cold and (est["pe"] + cold[0][1] + 2 * SC_MM
                                    <= est["act"] + GREEDY_M):
                        pop_cold_one()

                def oproj_st(st):
                    out = []
                    if True:
                        for nh in range(2):
                            def go(st=st, nh=nh):
                                tag = "qk" if nh == 0 else "vps"
                                y_ps = ps.tile([128, 512], f32, tag=tag,
                                               bufs=1, name=nm(f"psy{st}{nh}"))
                                for p in range(NPAIR):
                                    nc.tensor.matmul(
                                        y_ps,
                                        onorm[:, p, st * 128:(st + 1) * 128],
                                        wo_sb[:, p, nh * 512:(nh + 1) * 512],
                                        start=(p == 0), stop=(p == NPAIR - 1))
                                y_sb = wp.tile([128, 512], f32, tag="ysb",
                                               bufs=4, name=nm(f"ysb{st}{nh}"))
                                nc.vector.tensor_copy(y_sb, y_ps)
                                nc.sync.dma_start(
                                    out=y_d.ap()[st * 128:(st + 1) * 128,
                                                 nh * 512:(nh + 1) * 512],
                                    in_=y_sb)
                            out.append(("oproj", 880, go))
                    return out

                def attn_back(p, j, sq, att_t, last_slot):
                    """attn@v + normalize + transpose (+ copy & oproj when
                    this closes a (pair, sq) group)."""
                    def go():
                        qT, kT, vA = qkts[p]
                        o_ps = ps.tile([128, 4, 65], f32, tag="ops", bufs=1,
                                       name=nm(f"o{p}{j}{sq}"))
                        for sti in range(4):
                            for tt in range(NT):
                                nc.tensor.matmul(
                                    o_ps[:, sti, :],
                                    att_t[:, tt, sti * 128:(sti + 1) * 128],
                                    vA[:, tt, j, :],
                                    start=(tt == 0), stop=(tt == NT - 1))
                        rinv = wp.tile([128, 4], f32, tag="rinv", bufs=2,
                                       name=nm(f"ri{p}{j}{sq}"))
                        nc.vector.reciprocal(rinv, o_ps[:, :, 64])
                        o_n = wp.tile([128, 4, 64], bf16, tag="on", bufs=2,
                                      name=nm(f"on{p}{j}{sq}"))
                        for sti in range(4):
                            nc.vector.tensor_scalar_mul(
                                out=o_n[:, sti, :],
                                in0=o_ps[:, sti, 0:64],
                                scalar1=rinv[:, sti:sti + 1])
                        pt = pt_for[(p, sq)]
                        for sti in range(4):
                            nc.tensor.matmul(
                                pt[64 * j:64 * (j + 1), sti, :],
                                o_n[:, sti, :], eye, is_transpose=True)
                        if j == 1:
                            if last_slot:
                                for sti in range(4):
                                    st = sq * 4 + sti
                                    nc.vector.tensor_copy(
                                        onorm[:, p, st * 128:(st + 1) * 128],
                                        pt[:, sti, :])
                                    cold.extend(oproj_st(st))
                            else:
                                nc.vector.tensor_copy(
                                    onorm[:, p, sq * 512:(sq + 1) * 512],
                                    pt.rearrange("p st s -> p (st s)"))
                    return go

                def startup_pair0():
                    """kt-outer k-proj + q(sq0) + v(ttg0) emitted per
                    arriving x chunk; v ttg1-3 serial after; q sq1-3 to
                    cold. Attention(p0) can then start ~16us in."""
                    pair_tiles(0)
                    qT, kT, vA = qkts[0]
                    nc.gpsimd.memset(vA[:, :, :, 64:65], 1.0)
                    kps = [ps.tile([128, 2, 512], f32, tag="sc", bufs=2,
                                   name=nm(f"kp0{i}")) for i in range(2)]
                    q_ps = ps.tile([128, 512], f32, tag="qk", bufs=1,
                                   name=nm("psq00"))
                    v_ps = ps.tile([128, 512], f32, tag="vps", bufs=1,
                                   name=nm("psv00"))
                    v_ps4 = v_ps.rearrange("p (t m) -> p t m", t=4)
                    wk, wq, wv = (wtiles[("k", 0)], wtiles[("q", 0)],
                                  wtiles[("v", 0)])
                    for r in range(NKT + 3):
                        if r < NKT:
                            st_, sp_ = r == 0, r == NKT - 1
                            for sqh in range(NSQ):
                                nc.tensor.matmul(
                                    kps[sqh // 2][:, sqh % 2, :], wk[:, r, :],
                                    x_sb[:, r, sqh * 512:(sqh + 1) * 512],
                                    start=st_, stop=sp_)
                        if r >= 3:
                            kt = r - 3
                            st_, sp_ = kt == 0, kt == NKT - 1
                            nc.tensor.matmul(q_ps, wq[:, kt, :],
                                             x_sb[:, kt, 0:512],
                                             start=st_, stop=sp_)
                    # v groups must be sequential: a psum bank holds only one
                    # open accumulation group (start zeroes the bank region)
                    for t4 in range(4):
                        for kt in range(NKT):
                            nc.tensor.matmul(
                                v_ps4[:, t4, :],
                                x_sb[:, kt, t4 * 128:(t4 + 1) * 128],
                                wv[:, kt, :],
                                start=(kt == 0), stop=(kt == NKT - 1))
                    # startup bias-adds: k chunk for the first scores, then q,
                    # then the rest (DVE; keeps ACT's exp table resident)
                    nc.vector.tensor_scalar_add(
                        out=kT[:, 0:512], in0=kps[0][:, 0, :],
                        scalar1=bks[:, 0:1])
                    nc.vector.tensor_scalar_add(
                        out=qT[:, 0:512], in0=q_ps, scalar1=bqs[:, 0:1])
                    nc.vector.tensor_scalar_add(
                        out=kT[:, 512:1024], in0=kps[0][:, 1, :],
                        scalar1=bks[:, 0:1])
                    nc.vector.tensor_scalar_add(
                        out=kT[:, 1024:2048],
                        in0=kps[1].rearrange("p a b -> p (a b)"),
                        scalar1=bks[:, 0:1])
                    nc.vector.tensor_copy(
                        vA[:, 0:4, :, 0:64],
                        v_ps.rearrange("p (t j e) -> p t j e", t=4, j=2))
                    for a in v_atoms(0, 1):
                        a()
                    for ttg in (2, 3):
                        cold.extend(((0, "v"), 880, a)
                                    for a in v_atoms(0, ttg))
                    for sq in range(1, NSQ):
                        cold.extend(((0, f"q{sq}"), 880, a)
                                    for a in qk_atoms("q", 0, sq))
                    est["pe"] = 16000.0
                    est["act"] = 16000.0

                pt_for = {}

                w_dmas(0, only=("k",))
                if rep == 0:
                    nc.sync.dma_start(out=bqs, in_=bq_d.ap())
                    nc.sync.dma_start(out=bks, in_=bk_d.ap())
                    nc.sync.dma_start(out=eye, in_=eye_d.ap())
                for kt in range(1, 3):
                    nc.sync.dma_start(out=x_sb[:, kt, :], in_=xT_src[:, kt, :])
                w_dmas(0, only=("q", "v"))
                for kt in range(3, NKT):
                    nc.sync.dma_start(out=x_sb[:, kt, :], in_=xT_src[:, kt, :])
                if rep == 0:
                    nc.sync.dma_start(out=wo_sb, in_=wo_d.ap())
                startup_pair0()
                for slot in range(1, NPAIR + 1):
                    if slot < NPAIR:
                        w_dmas(slot)
                        atoms = qkv_atoms(slot)
                        if defer_q and slot == NPAIR - 1:
                            keep = [a for a in atoms
                                    if a[0][1] in ("v", "k", "q0")]
                            deferred = [a for a in atoms if a not in keep]
                            cold.extend(keep)
                        else:
                            cold.extend(atoms)
                    if defer_q and slot == NPAIR:
                        cold.extend(deferred)
                    p = slot - 1
                    last_slot = slot == NPAIR
                    for sq in range(NSQ):
                        pt_for[(p, sq)] = ps.tile(
                            [128, 4, 128], bf16, tag="pt", bufs=1,
                            name=nm(f"pt{p}{sq}"))
                        for j in range(2):
                            force_res((p, "k"), (p, f"q{sq}"))
                            qT, kT, vA = qkts[p]
                            att_t = wp.tile([128, NT, 512], bf16, tag="att",
                                            bufs=3, name=nm(f"att{p}{j}{sq}"))
                            lo = 64 * j
                            for g in range(NT // 2):
                                sc = ps.tile([128, 2, 512], f32, tag="sc",
                                             bufs=2,
                                             name=nm(f"sc{p}{j}{sq}{g}"))
                                dep = exp_hist[-2]
                                for i in range(2):
                                    tt = 2 * g + i
                                    nc.tensor.matmul(
                                        sc[:, i, :],
                                        kT[lo:lo + 64,
                                           tt * 128:(tt + 1) * 128],
                                        qT[lo:lo + 64,
                                           sq * 512:(sq + 1) * 512],
                                        start=True, stop=True)
                                est["pe"] = max(est["pe"] + 2 * SC_MM,
                                                dep + 150 + 2 * SC_MM)
                                nc.scalar.activation(
                                    att_t[:, 2 * g:2 * g + 2, :], sc,
                                    AF.Exp, scale=SCALE)
                                est["act"] = max(est["act"],
                                                 est["pe"] + 150) + EXP
                                exp_hist.append(est["act"])
                                if g == 3 and hot:
                                    hp, hfn = hot.popleft()
                                    force_res((hp, "v"))
                                    hfn()
                                    if len(exp_hist) >= 5:
                                        est["pe"] = max(est["pe"],
                                                        exp_hist[-5] + 150)
                                    est["pe"] += ATTNV + TRANSP
                                else:
                                    greedy_cold()
                            hot.append((p, attn_back(p, j, sq, att_t,
                                                     last_slot)))
                            if debug_taps and p == 0 and j == 0 and sq == 0:
                                def dbg0(att_t=att_t):
                                    force_res((0, "k"), (0, "v"), (0, "q1"),
                                              (0, "q2"), (0, "q3"))
                                    qT0, kT0, vA0 = qkts[0]
                                    nc.sync.dma_start(out=dbg_qT.ap(), in_=qT0)
                                    nc.sync.dma_start(out=dbg_kT.ap(), in_=kT0)
                                    nc.sync.dma_start(out=dbg_vA.ap(), in_=vA0)
                                    nc.sync.dma_start(out=dbg_att.ap(),
                                                      in_=att_t)
                                hot.append((0, dbg0))
                while hot:
                    hp, hfn = hot.popleft()
                    force_res((hp, "v"))
                    hfn()
                while cold:
                    pop_cold_one()
                if debug_taps:
                    nc.sync.dma_start(out=dbg_on.ap(), in_=onorm)
    nc.compile()
    return nc


def _get_nc():
    if "nc" not in _CACHE:
        _CACHE["nc"] = _build()
    return _CACHE["nc"]


def _bf16(a):
    import ml_dtypes
    return np.ascontiguousarray(a).astype(ml_dtypes.bfloat16)


def make_in_maps(x, Wq, Wk, Wv, bq, bk, bv, Wo, bo):
    eye = np.eye(128, dtype=np.float32)
    in_maps = []
    for c in range(NCORES):
        b, g = c // 2, c % 2
        hs = slice(g * HPC, (g + 1) * HPC)
        # weights pair-packed + sbuf-contiguous: [pair, 128pp, NKT, 128=(j,e)]
        def wprep(W):
            w3 = W[hs].reshape(NPAIR, 2, DIM, DH).transpose(0, 2, 1, 3) \
                .reshape(NPAIR, NKT, 128, 128).transpose(0, 2, 1, 3)
            return np.ascontiguousarray(w3)
        wq3, wk3, wv3 = wprep(Wq), wprep(Wk), wprep(Wv)
        # wo: [128=(j,e), pair, DIM]
        wo3 = Wo[g * 512:(g + 1) * 512, :].reshape(NPAIR, 128, DIM) \
            .transpose(1, 0, 2)
        in_maps.append({
            "xT": _bf16(x[b].T),
            "wq": _bf16(wq3),
            "wk": _bf16(wk3),
            "wv": _bf16(wv3),
            "bq": np.ascontiguousarray(bq[hs].reshape(NPAIR, 128).T),
            "bk": np.ascontiguousarray(bk[hs].reshape(NPAIR, 128).T),
            "wo": _bf16(wo3),
            "eye": _bf16(eye),
        })
    return in_maps


def combine(results, bv, Wo, bo):
    const = bv.reshape(DIM) @ Wo + bo          # [DIM]
    y = np.empty((B, S, DIM), dtype=np.float32)
    for b in range(B):
        y[b] = (results[2 * b]["y"].astype(np.float32)
                + results[2 * b + 1]["y"].astype(np.float32) + const)
    return y


def kernel(x, Wq, Wk, Wv, bq, bk, bv, Wo, bo):
    import time
    from concourse.bass_utils import run_bass_kernel_spmd
    x, Wq, Wk, Wv, bq, bk, bv, Wo, bo = [
        np.asarray(a, dtype=np.float32)
        for a in (x, Wq, Wk, Wv, bq, bk, bv, Wo, bo)]
    nc = _get_nc()
    in_maps = make_in_maps(x, Wq, Wk, Wv, bq, bk, bv, Wo, bo)
    last = None
    for attempt in range(3):
        try:
            res = run_bass_kernel_spmd(nc, in_maps,
                                       core_ids=list(range(NCORES)))
            return combine(res.results, bv, Wo, bo)
        except Exception as e:  # transient NRT_EXEC_UNIT_UNRECOVERABLE wedges
            last = e
            time.sleep(75)
    raise last


# revision 42
# speedup vs baseline: 1.3902x; 1.3902x over previous
"""Multi-head attention (BaselineAttention) Bass kernel for 8 trn2 NeuronCores.

Problem: x[4,2048,1024], per-head Wq/Wk/Wv [16,1024,64] (+biases), Wo[1024,1024]+bo.
Sharding: core c -> batch b=c//2, head-group g=c%2 (8 heads each).
Each core computes y_partial[b] = sum_{h in group} softmax(qk^T/8) v @ Wo_rows(h).
Host combines: y[b] = part[2b] + part[2b+1] + bo + bv@Wo  (bv folded out of device).

Device algorithm per core (all matmul operands bf16; psum f32):
  x resident in SBUF [128, 8kt, 2048]; per pair (2 heads): qT/kT [128=(j,e), s]
  via W^T@x; v[t,(j,e)] via x^T@Wv with an appended ones column (vA [.,tt,j,65]).
  Attention per (head, s-chunk 512): scores^T[t,s] = kT^T qT (K=64); exp on ACT
  -> att bf16 [t, s]; o_aug[s, e|r] = att^T-stationary @ [v|1]-moving (N=65,
  psum-accumulated over 16 t-tiles); normalize with DVE per-partition scalar
  1/r; PE-transpose o_n -> onorm[(j,e), pair, s]; out-proj y = onorm^T @ Wo.
  Pair-pipelined schedule: QKV of pair p+1 and out-proj fill PE while ACT
  computes exp of pair p (exp is the co-bottleneck at ~267us vs PE ~285us).
"""
import numpy as np

B, S, DIM, H, DH = 4, 2048, 1024, 16, 64
NCORES = 8
HPC = H // 2          # heads per core = 8
NPAIR = HPC // 2      # head pairs per core = 4
NT = S // 128         # t-tiles = 16
NSQ = S // 512        # s-chunks of 512 = 4
NKT = DIM // 128      # d-tiles = 8
SCALE = 1.0 / float(np.sqrt(DH))

_CACHE = {}


def _build(repeat=1, debug_taps=False, exp_ns=1040, attnv_ns=1830, defer_q=True, greedy_m=-600):
    from collections import deque
    import concourse.bass as bass  # noqa: F401
    import concourse.mybir as mybir
    import concourse.tile as tile
    from concourse import bacc

    f32 = mybir.dt.float32
    bf16 = mybir.dt.bfloat16
    AF = mybir.ActivationFunctionType

    nc = bacc.Bacc("TRN2", target_bir_lowering=False, debug=False,
                   num_devices=NCORES)

    xT_d = nc.dram_tensor("xT", [DIM, S], bf16, kind="ExternalInput")
    wq_d = nc.dram_tensor("wq", [NPAIR, 128, NKT, 128], bf16, kind="ExternalInput")
    wk_d = nc.dram_tensor("wk", [NPAIR, 128, NKT, 128], bf16, kind="ExternalInput")
    wv_d = nc.dram_tensor("wv", [NPAIR, 128, NKT, 128], bf16, kind="ExternalInput")
    bq_d = nc.dram_tensor("bq", [128, NPAIR], f32, kind="ExternalInput")
    bk_d = nc.dram_tensor("bk", [128, NPAIR], f32, kind="ExternalInput")
    wo_d = nc.dram_tensor("wo", [128, NPAIR, DIM], bf16, kind="ExternalInput")
    eye_d = nc.dram_tensor("eye", [128, 128], bf16, kind="ExternalInput")
    y_d = nc.dram_tensor("y", [S, DIM], f32, kind="ExternalOutput")
    if debug_taps:
        dbg_qT = nc.dram_tensor("dbg_qT", [128, S], bf16, kind="ExternalOutput")
        dbg_kT = nc.dram_tensor("dbg_kT", [128, S], bf16, kind="ExternalOutput")
        dbg_vA = nc.dram_tensor("dbg_vA", [128, NT, 2, 65], bf16,
                                kind="ExternalOutput")
        dbg_att = nc.dram_tensor("dbg_att", [128, NT, 512], bf16,
                                 kind="ExternalOutput")
        dbg_on = nc.dram_tensor("dbg_on", [128, NPAIR, S], bf16,
                                kind="ExternalOutput")

    ctr = [0]

    def nm(pfx):
        ctr[0] += 1
        return f"{pfx}_{ctr[0]}"

    with tile.TileContext(nc) as tc:
        with tc.tile_pool(name="persist", bufs=1) as pp, \
             tc.tile_pool(name="work", bufs=1) as wp, \
             tc.tile_pool(name="ps", bufs=1, space="PSUM") as ps:
            # ---- persistent SBUF ----
            x_sb = pp.tile([128, NKT, S], bf16)
            eye = pp.tile([128, 128], bf16)
            bqs = pp.tile([128, NPAIR], f32)
            bks = pp.tile([128, NPAIR], f32)
            wo_sb = pp.tile([128, NPAIR, DIM], bf16)
            onorm = pp.tile([128, NPAIR, S], bf16)   # [(j,e), pair, s]

            for rep in range(repeat):
                xT_src = xT_d.ap().rearrange("(kt p) s -> p kt s", p=128)
                nc.sync.dma_start(out=x_sb[:, 0, :], in_=xT_src[:, 0, :])

                wtiles = {}

                def w_dmas(p, only=None):
                    for pfx, wd in (("k", wk_d), ("q", wq_d), ("v", wv_d)):
                        if only and pfx not in only:
                            continue
                        w = wp.tile([128, NKT, 128], bf16, tag=f"w{pfx}",
                                    bufs=2, name=nm(f"w{pfx}{p}"))
                        nc.sync.dma_start(out=w, in_=wd.ap()[p])
                        wtiles[(pfx, p)] = w

                qkts = {}

                def pair_tiles(p):
                    qT = wp.tile([128, S], bf16, tag="qT", bufs=2,
                                 name=nm(f"qT{p}"))
                    kT = wp.tile([128, S], bf16, tag="kT", bufs=2,
                                 name=nm(f"kT{p}"))
                    vA = wp.tile([128, NT, 2, 65], bf16, tag="vA", bufs=2,
                                 name=nm(f"vA{p}"))
                    qkts[p] = (qT, kT, vA)

                def ones_atom(p):
                    def go():
                        nc.gpsimd.memset(qkts[p][2][:, :, :, 64:65], 1.0)
                    return go

                def qk_atoms(pfx, p, sq):
                    """Two ~850ns PE atoms (4 kt-matmuls each) + bias copy."""
                    cell = {}

                    def a1():
                        dst, bias = ((qkts[p][0], bqs) if pfx == "q"
                                     else (qkts[p][1], bks))
                        cell["ps"] = ps.tile([128, 512], f32, tag="qk",
                                             bufs=1, name=nm(f"ps{pfx}{p}{sq}"))
                        cell["dst"], cell["bias"] = dst, bias
                        w = wtiles[(pfx, p)]
                        for kt in range(4):
                            nc.tensor.matmul(
                                cell["ps"], w[:, kt, :],
                                x_sb[:, kt, sq * 512:(sq + 1) * 512],
                                start=(kt == 0), stop=False)

                    def a2():
                        w = wtiles[(pfx, p)]
                        for kt in range(4, NKT):
                            nc.tensor.matmul(
                                cell["ps"], w[:, kt, :],
                                x_sb[:, kt, sq * 512:(sq + 1) * 512],
                                start=False, stop=(kt == NKT - 1))
                        nc.vector.tensor_scalar_add(
                            out=cell["dst"][:, sq * 512:(sq + 1) * 512],
                            in0=cell["ps"], scalar1=cell["bias"][:, p:p + 1])
                    return [a1, a2]

                def v_atoms(p, ttg):
                    """Two ~850ns PE atoms (2 t-tiles each) + copy to vA."""
                    cell = {}

                    def half(h):
                        def go():
                            if h == 0:
                                cell["ps"] = ps.tile(
                                    [128, 512], f32, tag="vps", bufs=1,
                                    name=nm(f"psv{p}{ttg}"))
                            v_ps4 = cell["ps"].rearrange("p (t m) -> p t m",
                                                         t=4)
                            w = wtiles[("v", p)]
                            for t4 in (0, 1) if h == 0 else (2, 3):
                                tt = ttg * 4 + t4
                                for kt in range(NKT):
                                    nc.tensor.matmul(
                                        v_ps4[:, t4, :],
                                        x_sb[:, kt, tt * 128:(tt + 1) * 128],
                                        w[:, kt, :],
                                        start=(kt == 0), stop=(kt == NKT - 1))
                            if h == 1:
                                vA = qkts[p][2]
                                nc.vector.tensor_copy(
                                    vA[:, ttg * 4:(ttg + 1) * 4, :, 0:64],
                                    cell["ps"].rearrange(
                                        "p (t j e) -> p t j e", t=4, j=2))
                        return go
                    return [half(0), half(1)]

                def qkv_atoms(p):
                    """(resource_key, cost_ns, closure) atoms for pair p."""
                    pair_tiles(p)
                    out = [((p, "v"), 100, ones_atom(p))]
                    for ttg in range(4):
                        out += [((p, "v"), 880, a) for a in v_atoms(p, ttg)]
                    for sq in range(NSQ):
                        out += [((p, "k"), 880, a)
                                for a in qk_atoms("k", p, sq)]
                    for sq in range(NSQ):
                        out += [((p, f"q{sq}"), 880, a)
                                for a in qk_atoms("q", p, sq)]
                    return out

                # ---- greedy emission scheduler ----
                # Estimated engine clocks (ns) maintained during emission;
                # cold filler (QKV / out-proj atoms) is emitted only while
                # it cannot starve ACT of score tiles.
                GREEDY_M = greedy_m
                SC_MM = 217       # one [128,512] scores matmul
                EXP = exp_ns      # one [128,1024] exp + init
                ATTNV = attnv_ns  # 64 x 65-col matmuls
                TRANSP = 320
                est = {"pe": 0.0, "act": 0.0}
                exp_hist = [0.0, 0.0]   # completion time per sc-tile rotation

                hot = deque()
                cold = deque()

                def pop_cold_one():
                    key, cost, fn = cold.popleft()
                    fn()
                    est["pe"] += cost

                def force_res(*keys):
                    """Emit every cold atom up to and incl. the given
                    resource keys (FIFO order preserved)."""
                    while any(k for (k, c, f) in cold if k in keys):
                        pop_cold_one()

                def greedy_cold():
                    while cold and (est["pe"] + cold[0][1] + 2 * SC_MM
                                    <= est["act"] + GREEDY_M):
                        pop_cold_one()

                def oproj_st(st):
                    out = []
                    if True:
                        for nh in range(2):
                            def go(st=st, nh=nh):
                                tag = "qk" if nh == 0 else "vps"
                                y_ps = ps.tile([128, 512], f32, tag=tag,
                                               bufs=1, name=nm(f"psy{st}{nh}"))
                                for p in range(NPAIR):
                                    nc.tensor.matmul(
                                        y_ps,
                                        onorm[:, p, st * 128:(st + 1) * 128],
                                        wo_sb[:, p, nh * 512:(nh + 1) * 512],
                                        start=(p == 0), stop=(p == NPAIR - 1))
                                y_sb = wp.tile([128, 512], f32, tag="ysb",
                                               bufs=4, name=nm(f"ysb{st}{nh}"))
                                nc.vector.tensor_copy(y_sb, y_ps)
                                nc.sync.dma_start(
                                    out=y_d.ap()[st * 128:(st + 1) * 128,
                                                 nh * 512:(nh + 1) * 512],
                                    in_=y_sb)
                            out.append(("oproj", 880, go))
                    return out

                def attn_back(p, j, sq, att_t, last_slot):
                    """attn@v + normalize + transpose (+ copy & oproj when
                    this closes a (pair, sq) group)."""
                    def go():
                        qT, kT, vA = qkts[p]
                        o_ps = ps.tile([128, 4, 65], f32, tag="ops", bufs=1,
                                       name=nm(f"o{p}{j}{sq}"))
                        for sti in range(4):
                            for tt in range(NT):
                                nc.tensor.matmul(
                                    o_ps[:, sti, :],
                                    att_t[:, tt, sti * 128:(sti + 1) * 128],
                                    vA[:, tt, j, :],
                                    start=(tt == 0), stop=(tt == NT - 1))
                        rinv = wp.tile([128, 4], f32, tag="rinv", bufs=2,
                                       name=nm(f"ri{p}{j}{sq}"))
                        nc.vector.reciprocal(rinv, o_ps[:, :, 64])
                        o_n = wp.tile([128, 4, 64], bf16, tag="on", bufs=2,
                                      name=nm(f"on{p}{j}{sq}"))
                        for sti in range(4):
                            nc.vector.tensor_scalar_mul(
                                out=o_n[:, sti, :],
                                in0=o_ps[:, sti, 0:64],
                                scalar1=rinv[:, sti:sti + 1])
                        pt = pt_for[(p, sq)]
                        for sti in range(4):
                            nc.tensor.matmul(
                                pt[64 * j:64 * (j + 1), sti, :],
                                o_n[:, sti, :], eye, is_transpose=True)
                        if j == 1:
                            if last_slot:
                                for sti in range(4):
                                    st = sq * 4 + sti
                                    nc.vector.tensor_copy(
                                        onorm[:, p, st * 128:(st + 1) * 128],
                                        pt[:, sti, :])
                                    cold.extend(oproj_st(st))
                            else:
                                nc.vector.tensor_copy(
                                    onorm[:, p, sq * 512:(sq + 1) * 512],
                                    pt.rearrange("p st s -> p (st s)"))
                    return go

                def startup_pair0():
                    """kt-outer k-proj + q(sq0) + v(ttg0) emitted per
                    arriving x chunk; v ttg1-3 serial after; q sq1-3 to
                    cold. Attention(p0) can then start ~16us in."""
                    pair_tiles(0)
                    qT, kT, vA = qkts[0]
                    nc.gpsimd.memset(vA[:, :, :, 64:65], 1.0)
                    kps = [ps.tile([128, 2, 512], f32, tag="sc", bufs=2,
                                   name=nm(f"kp0{i}")) for i in range(2)]
                    q_ps = ps.tile([128, 512], f32, tag="qk", bufs=1,
                                   name=nm("psq00"))
                    wk, wq, wv = (wtiles[("k", 0)], wtiles[("q", 0)],
                                  wtiles[("v", 0)])
                    for r in range(NKT + 3):
                        if r < NKT:
                            st_, sp_ = r == 0, r == NKT - 1
                            for sqh in range(NSQ):
                                nc.tensor.matmul(
                                    kps[sqh // 2][:, sqh % 2, :], wk[:, r, :],
                                    x_sb[:, r, sqh * 512:(sqh + 1) * 512],
                                    start=st_, stop=sp_)
                        if r >= 3:
                            kt = r - 3
                            st_, sp_ = kt == 0, kt == NKT - 1
                            nc.tensor.matmul(q_ps, wq[:, kt, :],
                                             x_sb[:, kt, 0:512],
                                             start=st_, stop=sp_)

                    # startup bias-adds: k chunk for the first scores, then q,
                    # then the rest (DVE; keeps ACT's exp table resident)
                    nc.vector.tensor_scalar_add(
                        out=kT[:, 0:512], in0=kps[0][:, 0, :],
                        scalar1=bks[:, 0:1])
                    nc.vector.tensor_scalar_add(
                        out=qT[:, 0:512], in0=q_ps, scalar1=bqs[:, 0:1])
                    nc.vector.tensor_scalar_add(
                        out=kT[:, 512:1024], in0=kps[0][:, 1, :],
                        scalar1=bks[:, 0:1])
                    nc.vector.tensor_scalar_add(
                        out=kT[:, 1024:2048],
                        in0=kps[1].rearrange("p a b -> p (a b)"),
                        scalar1=bks[:, 0:1])
                    for ttg in (0, 1, 2, 3):
                        cold.extend(((0, "v"), 880, a)
                                    for a in v_atoms(0, ttg))
                    for sq in range(1, NSQ):
                        cold.extend(((0, f"q{sq}"), 880, a)
                                    for a in qk_atoms("q", 0, sq))
                    est["pe"] = 16000.0
                    est["act"] = 16000.0

                pt_for = {}

                w_dmas(0, only=("k",))
                if rep == 0:
                    nc.sync.dma_start(out=bqs, in_=bq_d.ap())
                    nc.sync.dma_start(out=bks, in_=bk_d.ap())
                    nc.sync.dma_start(out=eye, in_=eye_d.ap())
                for kt in range(1, 3):
                    nc.sync.dma_start(out=x_sb[:, kt, :], in_=xT_src[:, kt, :])
                w_dmas(0, only=("q", "v"))
                for kt in range(3, NKT):
                    nc.sync.dma_start(out=x_sb[:, kt, :], in_=xT_src[:, kt, :])
                if rep == 0:
                    nc.sync.dma_start(out=wo_sb, in_=wo_d.ap())
                startup_pair0()
                for slot in range(1, NPAIR + 1):
                    if slot < NPAIR:
                        w_dmas(slot)
                        atoms = qkv_atoms(slot)
                        if defer_q and slot == NPAIR - 1:
                            keep = [a for a in atoms
                                    if a[0][1] in ("v", "k", "q0")]
                            deferred = [a for a in atoms if a not in keep]
                            cold.extend(keep)
                        else:
                            cold.extend(atoms)
                    if defer_q and slot == NPAIR:
                        cold.extend(deferred)
                    p = slot - 1
                    last_slot = slot == NPAIR
                    for sq in range(NSQ):
                        pt_for[(p, sq)] = ps.tile(
                            [128, 4, 128], bf16, tag="pt", bufs=1,
                            name=nm(f"pt{p}{sq}"))
                        for j in range(2):
                            force_res((p, "k"), (p, f"q{sq}"))
                            qT, kT, vA = qkts[p]
                            att_t = wp.tile([128, NT, 512], bf16, tag="att",
                                            bufs=3, name=nm(f"att{p}{j}{sq}"))
                            lo = 64 * j
                            for g in range(NT // 2):
                                sc = ps.tile([128, 2, 512], f32, tag="sc",
                                             bufs=2,
                                             name=nm(f"sc{p}{j}{sq}{g}"))
                                dep = exp_hist[-2]
                                for i in range(2):
                                    tt = 2 * g + i
                                    nc.tensor.matmul(
                                        sc[:, i, :],
                                        kT[lo:lo + 64,
                                           tt * 128:(tt + 1) * 128],
                                        qT[lo:lo + 64,
                                           sq * 512:(sq + 1) * 512],
                                        start=True, stop=True)
                                est["pe"] = max(est["pe"] + 2 * SC_MM,
                                                dep + 150 + 2 * SC_MM)
                                nc.scalar.activation(
                                    att_t[:, 2 * g:2 * g + 2, :], sc,
                                    AF.Exp, scale=SCALE)
                                est["act"] = max(est["act"],
                                                 est["pe"] + 150) + EXP
                                exp_hist.append(est["act"])
                                if g == 3 and hot:
                                    hp, hfn = hot.popleft()
                                    force_res((hp, "v"))
                                    hfn()
                                    if len(exp_hist) >= 5:
                                        est["pe"] = max(est["pe"],
                                                        exp_hist[-5] + 150)
                                    est["pe"] += ATTNV + TRANSP
                                else:
                                    greedy_cold()
                            hot.append((p, attn_back(p, j, sq, att_t,
                                                     last_slot)))
                            if debug_taps and p == 0 and j == 0 and sq == 0:
                                def dbg0(att_t=att_t):
                                    force_res((0, "k"), (0, "v"), (0, "q1"),
                                              (0, "q2"), (0, "q3"))
                                    qT0, kT0, vA0 = qkts[0]
                                    nc.sync.dma_start(out=dbg_qT.ap(), in_=qT0)
                                    nc.sync.dma_start(out=dbg_kT.ap(), in_=kT0)
                                    nc.sync.dma_start(out=dbg_vA.ap(), in_=vA0)
                                    nc.sync.dma_start(out=dbg_att.ap(),
                                                      in_=att_t)
                                hot.append((0, dbg0))
                while hot:
                    hp, hfn = hot.popleft()
                    force_res((hp, "v"))
                    hfn()
                while cold:
                    pop_cold_one()
                if debug_taps:
                    nc.sync.dma_start(out=dbg_on.ap(), in_=onorm)
    nc.compile()
    return nc


def _get_nc():
    if "nc" not in _CACHE:
        _CACHE["nc"] = _build()
    return _CACHE["nc"]


def _bf16(a):
    import ml_dtypes
    return np.ascontiguousarray(a).astype(ml_dtypes.bfloat16)


def make_in_maps(x, Wq, Wk, Wv, bq, bk, bv, Wo, bo):
    eye = np.eye(128, dtype=np.float32)
    in_maps = []
    for c in range(NCORES):
        b, g = c // 2, c % 2
        hs = slice(g * HPC, (g + 1) * HPC)
        # weights pair-packed + sbuf-contiguous: [pair, 128pp, NKT, 128=(j,e)]
        def wprep(W):
            w3 = W[hs].reshape(NPAIR, 2, DIM, DH).transpose(0, 2, 1, 3) \
                .reshape(NPAIR, NKT, 128, 128).transpose(0, 2, 1, 3)
            return np.ascontiguousarray(w3)
        wq3, wk3, wv3 = wprep(Wq), wprep(Wk), wprep(Wv)
        # wo: [128=(j,e), pair, DIM]
        wo3 = Wo[g * 512:(g + 1) * 512, :].reshape(NPAIR, 128, DIM) \
            .transpose(1, 0, 2)
        in_maps.append({
            "xT": _bf16(x[b].T),
            "wq": _bf16(wq3),
            "wk": _bf16(wk3),
            "wv": _bf16(wv3),
            "bq": np.ascontiguousarray(bq[hs].reshape(NPAIR, 128).T),
            "bk": np.ascontiguousarray(bk[hs].reshape(NPAIR, 128).T),
            "wo": _bf16(wo3),
            "eye": _bf16(eye),
        })
    return in_maps


def combine(results, bv, Wo, bo):
    const = bv.reshape(DIM) @ Wo + bo          # [DIM]
    y = np.empty((B, S, DIM), dtype=np.float32)
    for b in range(B):
        y[b] = (results[2 * b]["y"].astype(np.float32)
                + results[2 * b + 1]["y"].astype(np.float32) + const)
    return y


def kernel(x, Wq, Wk, Wv, bq, bk, bv, Wo, bo):
    import time
    from concourse.bass_utils import run_bass_kernel_spmd
    x, Wq, Wk, Wv, bq, bk, bv, Wo, bo = [
        np.asarray(a, dtype=np.float32)
        for a in (x, Wq, Wk, Wv, bq, bk, bv, Wo, bo)]
    nc = _get_nc()
    in_maps = make_in_maps(x, Wq, Wk, Wv, bq, bk, bv, Wo, bo)
    last = None
    for attempt in range(3):
        try:
            res = run_bass_kernel_spmd(nc, in_maps,
                                       core_ids=list(range(NCORES)))
            return combine(res.results, bv, Wo, bo)
        except Exception as e:  # transient NRT_EXEC_UNIT_UNRECOVERABLE wedges
            last = e
            time.sleep(75)
    raise last
